# revision 11
# baseline (speedup 1.0000x reference)
"""AuroraAttention Trainium2 kernel — 8-core SPMD, head-sharded, v3.

Strategy (tensor parallel over heads, per sharding hint):
  - 16 heads -> 2 heads per core; both batches on every core.
  - Per core: q/k/v projections restricted to its 2 heads (column-parallel),
    full attention for its (batch, head) pairs, row-parallel output
    projection producing a partial [B, S, E] output; host sums the 8
    partials.
  - Scores are computed TRANSPOSED (S^T[k, q]) so the attention-weight
    matrix is already laid out with the contraction dim (k) on partitions
    for the A@V matmul. A 64-wide ones block in the V operand makes the
    same matmul produce the softmax denominators already broadcast across
    64 partitions.
  - softmax(s + b) = exp(s) * exp(b) with exp(b) precomputed on the host
    in bf16.

v3 performance structure (v1 ~383us, v2 ~404us):
  - ACT (scalar engine) is the long pole: the 16.8M-element exp stream is
    ~1ns/col + ~250-400ns fixed overhead per instruction.  So exps stay
    fused at [128, 1024] (both heads of one (kt, batch) pair -> 128 ops
    total), and EVERYTHING else is kept off ACT during attention.
  - Software pipeline with per-(kt,b) PSUM score tiles (2 banks, pool of
    2) and the drain chain (exp -> bias-multiply -> AV matmuls) emitted
    two sub-iterations behind the score matmuls: by drain time the
    dependencies have cleared, so the PE stream never parks on the
    serial exp->mul path (stalls also drop the PE out of its 2.4GHz
    p-state, which is what made v1/v2 matmuls ~1.7x slow).
  - 1/sumexp via DVE reciprocal_approx_fast (18-bit accurate, ~5x faster
    than the 4us InstReciprocal v1 used).
  - Every 4th bias-multiply runs on the otherwise-idle GPSIMD engine.
  - Output projection (one [128 s, 512 e] chunk at a time, PSUM tiles
    borrowed from the score pool, staged through SBUF on DVE) is
    deferred one q-block and spliced every other sub-iteration.
  - exp's ACT table is warmed up during the projection phase.
"""

import numpy as np
import ml_dtypes

import concourse.bass as bass
import concourse.mybir as mybir
import concourse.tile as tile
from concourse.bass_utils import run_bass_kernel_spmd
from concourse.masks import make_identity
from bass_rust import SyncInfo

BF16 = ml_dtypes.bfloat16
F32 = mybir.dt.float32
BF = mybir.dt.bfloat16

H, D, B, S, E = 16, 64, 2, 2048, 1024
N_CORES = 8
HPC = H // N_CORES  # heads per core
NQB = S // 512  # 4 q blocks
NKT = S // 128  # 16 k tiles
ECH = E // 128  # 8 contraction chunks for projections

SKEW = 2  # drain chain lags the score matmuls by this many (kt, b) sub-iters

# ---------------------------------------------------------------------------
# This walrus build rejects instructions carrying more than one sem wait
# ("Too many sync wait commands"). Tile freely emits multi-wait
# instructions, so after scheduling we move extra waits onto same-engine
# NoOps inserted immediately before the affected instruction. Engine
# streams execute in program order, so waiting on a preceding NoOp is
# semantically identical to waiting on the instruction itself.
_MAX_WAITS = 1


def split_multi_waits(nc: bass.Bass, max_waits: int = _MAX_WAITS):
    for bb in nc.main_func.blocks:
        lst = bb.instructions
        new = []
        changed = False
        for inst in lst:
            si = inst.sync_info
            if si is not None and si.on_wait and len(si.on_wait) > max_waits:
                waits = list(si.on_wait)
                extra, keep = waits[:-max_waits], waits[-max_waits:]
                for i in range(0, len(extra), max_waits):
                    nop = mybir.InstNoOp(
                        name=nc.get_next_instruction_name(), ins=[], outs=[]
                    )
                    nop.engine = inst.engine
                    nop.sync_info = SyncInfo(
                        on_wait=extra[i : i + max_waits], on_update=[]
                    )
                    nc.register_instruction(nop)
                    new.append(nop)
                inst.sync_info = SyncInfo(on_wait=keep, on_update=si.on_update)
                changed = True
            new.append(inst)
        if changed:
            bb.instructions = new
# ---------------------------------------------------------------------------


def build_nc() -> bass.Bass:
    nc = bass.Bass()

    # partition-major host layouts so each load is one big DMA
    xt = nc.dram_tensor("xt", [B, 128, ECH, S], BF, kind="ExternalInput")
    wq = nc.dram_tensor("wq", [128, ECH, 128], BF, kind="ExternalInput")
    wk = nc.dram_tensor("wk", [128, ECH, 128], BF, kind="ExternalInput")
    wv = nc.dram_tensor("wv", [128, ECH, 128], BF, kind="ExternalInput")
    bqkv = nc.dram_tensor("bqkv", [128, 3], F32, kind="ExternalInput")
    wo = nc.dram_tensor("wo", [128, E], BF, kind="ExternalInput")
    # exp(bias) transposed + host-packed so one [128, 1024] tile covering both
    # heads is one contiguous DMA: pbias[k, qb, h, q'] = exp(bias[0, h, qb*512+q', k])
    pbias = nc.dram_tensor("pbias", [S, NQB, HPC, 512], BF, kind="ExternalInput")
    out = nc.dram_tensor("out", [B, S, E], BF, kind="ExternalOutput")

    with tile.TileContext(nc) as tc:
        _emit(tc, nc, xt, wq, wk, wv, bqkv, wo, pbias, out)
    split_multi_waits(nc)
    return nc


def _emit(tc, nc, xt, wq, wk, wv, bqkv, wo, pbias, out):
    with tc.tile_pool(name="persist", bufs=1) as persist:
        # ---- persistent SBUF tensors -----------------------------------
        xt_sb = persist.tile([128, B, ECH, S], BF)  # hidden^T
        w_sb = persist.tile([128, 3, ECH, 128], BF)  # WqT/WkT/WvT chunks
        b_sb = persist.tile([128, 3], F32)  # bq/bk/bv (prescaled)
        wo_sb = persist.tile([128, E], BF)  # Wo slice^T, both heads
        qT_sb = persist.tile([128, B, S], BF)  # q^T (2 heads on partitions)
        kT_sb = persist.tile([128, B, S], BF)
        vT_sb = persist.tile([128, B, S], BF)  # v^T before transpose
        # v natural layout per k-tile: [v_h0 | ones64 | ones64 | v_h1]
        # -> AV matmul h0 gives O^T rows 0:64 + bcast sums rows 64:128;
        #    AV matmul h1 gives bcast sums rows 0:64 + O^T rows 64:128.
        v_sb = persist.tile([128, B, NKT, 256], BF)
        o_norm = persist.tile([128, B, S], BF)  # normalized O^T, both heads
        ident = persist.tile([128, 128], BF)
        warm = persist.tile([128, 1], F32)

        nc.vector.memset(v_sb[:, :, :, 64:192], 1.0)
        nc.vector.memset(warm, 0.0)
        # load exp's ACT table during the projection phase, not at the
        # first real exp
        nc.scalar.activation(
            out=warm, in_=warm, func=mybir.ActivationFunctionType.Exp
        )
        make_identity(nc, ident)

        # batched initial loads (host layouts are partition-major)
        for pi, w in enumerate((wq, wk, wv)):
            nc.sync.dma_start(out=w_sb[:, pi, :, :], in_=w[:, :, :])
        nc.sync.dma_start(out=b_sb, in_=bqkv[:, :])
        nc.sync.dma_start(out=wo_sb, in_=wo[:, :])
        for b in range(B):
            nc.sync.dma_start(out=xt_sb[:, b, :, :], in_=xt[b])

        # ---- projections ------------------------------------------------
        with (
            tc.tile_pool(name="proj_ps", bufs=2, space="PSUM") as proj_ps,
            tc.tile_pool(name="vtr_ps", bufs=2, space="PSUM") as vtr_ps,
        ):
            dsts = (qT_sb, kT_sb, vT_sb)
            for b in range(B):
                for pi in range(3):
                    for sblk in range(S // 512):
                        ps = proj_ps.tile([128, 512], F32, name="pps")
                        for c in range(ECH):
                            nc.tensor.matmul(
                                ps,
                                lhsT=w_sb[:, pi, c, :],
                                rhs=xt_sb[:, b, c, sblk * 512 : (sblk + 1) * 512],
                                start=(c == 0),
                                stop=(c == ECH - 1),
                            )
                        nc.scalar.activation(
                            out=dsts[pi][:, b, sblk * 512 : (sblk + 1) * 512],
                            in_=ps,
                            func=mybir.ActivationFunctionType.Identity,
                            bias=b_sb[:, pi : pi + 1],
                            scale=1.0,
                        )
                # v^T -> v natural (PE transpose per 128-wide s tile)
                for st in range(NKT):
                    tp = vtr_ps.tile([128, 128], BF, name="vtp")
                    nc.tensor.transpose(
                        out=tp,
                        in_=vT_sb[:, b, st * 128 : (st + 1) * 128],
                        identity=ident,
                    )
                    nc.scalar.copy(out=v_sb[:, b, st, 0:64], in_=tp[:, 0:64])
                    nc.scalar.copy(out=v_sb[:, b, st, 192:256], in_=tp[:, 64:128])

        # ---- software-pipelined attention -------------------------------
        # Sub-iteration = (kt, b): two K=64 score matmuls (row-packed heads)
        # into one 2-bank PSUM tile, one fused exp on ACT, one bias
        # multiply (DVE, every 4th on GPSIMD), two AV matmuls accumulating
        # into oacc[b][h].  The drain chain runs SKEW sub-iters behind the
        # score matmuls.  ps_pool (2 bufs x 2 banks) rotates score tiles
        # and the borrowed output-projection tiles; oacc holds the other
        # 4 banks.
        with (
            tc.tile_pool(name="eb_sb", bufs=4) as eb_pool,
            tc.tile_pool(name="pt_sb", bufs=4) as pt_pool,
            tc.tile_pool(name="norm_sb", bufs=4) as norm_pool,
            tc.tile_pool(name="wo_stage", bufs=4) as wo_stage,
            tc.tile_pool(name="ps_pool", bufs=2, space="PSUM") as ps_pool,
            tc.tile_pool(name="oacc_ps", bufs=1, space="PSUM") as oacc_ps,
        ):
            PREF = 3  # ebt DMA prefetch distance in kt units

            def norm_chunk(qb, b, h, oacc_t):
                # o_norm = O^T / sumexp; ones-block placement puts
                # h0: O^T rows 0:64, sums rows 64:128 (h1 mirrored).
                # 1/x on ACT as exp(-ln(x)): Ln and Exp share an act
                # table, and ACT has slack while DVE (where v1's 4us
                # reciprocal lived) is the attention-phase pacer.
                qs = slice(qb * 512, (qb + 1) * 512)
                o_rows = slice(0, 64) if h == 0 else slice(64, 128)
                s_rows = slice(64, 128) if h == 0 else slice(0, 64)
                ln_t = norm_pool.tile([128, 512], F32, name=f"ln{b}{h}")
                r_t = norm_pool.tile([128, 512], F32, name=f"r{b}{h}")
                nc.scalar.activation(
                    out=ln_t[s_rows, :],
                    in_=oacc_t[s_rows, :],
                    func=mybir.ActivationFunctionType.Ln,
                )
                nc.scalar.activation(
                    out=r_t[o_rows, :],
                    in_=ln_t[s_rows, :],
                    func=mybir.ActivationFunctionType.Exp,
                    scale=-1.0,
                )
                nc.vector.tensor_mul(
                    out=o_norm[o_rows, b, qs],
                    in0=oacc_t[o_rows, :],
                    in1=r_t[o_rows, :],
                )

            def wo_chunk(qb, b, sti, eb):
                # one [128 s, 512 e] slice of the output projection,
                # staged through SBUF on DVE (no PSUM->DRAM DMA on this
                # build)
                st = qb * 4 + sti
                ps = ps_pool.tile([128, 512], F32, name="ps")
                nc.tensor.matmul(
                    ps,
                    lhsT=o_norm[:, b, st * 128 : (st + 1) * 128],
                    rhs=wo_sb[:, eb * 512 : (eb + 1) * 512],
                    start=True,
                    stop=True,
                )
                stg = wo_stage.tile([128, 512], BF, name="stg")
                nc.vector.tensor_copy(out=stg, in_=ps)
                nc.sync.dma_start(
                    out=out[b, st * 128 : (st + 1) * 128, eb * 512 : (eb + 1) * 512],
                    in_=stg,
                )

            subs = [(kt, b) for kt in range(NKT) for b in range(B)]
            NSUB = len(subs)  # 32
            pending: list = []  # deferred wo chunks of the previous qb
            for qb in range(NQB):
                qs = slice(qb * 512, (qb + 1) * 512)
                oacc = [
                    [
                        oacc_ps.tile([128, 512], F32, name=f"oacc_{b}_{h}")
                        for h in range(HPC)
                    ]
                    for b in range(B)
                ]
                ebt = {}
                for kt in range(PREF):
                    t = eb_pool.tile([128, 1024], BF, name="ebt")
                    nc.sync.dma_start(out=t, in_=pbias[kt * 128 : (kt + 1) * 128, qb])
                    ebt[kt] = t
                stile = {}
                for i in range(NSUB + SKEW):
                    if i < NSUB:
                        kt, b = subs[i]
                        if b == 0 and kt + PREF < NKT:
                            pkt = kt + PREF
                            t = eb_pool.tile([128, 1024], BF, name="ebt")
                            nc.sync.dma_start(
                                out=t, in_=pbias[pkt * 128 : (pkt + 1) * 128, qb]
                            )
                            ebt[pkt] = t
                        ks = slice(kt * 128, (kt + 1) * 128)
                        s_t = ps_pool.tile([128, 1024], F32, name="ps")
                        for h in range(HPC):
                            hp = slice(h * 64, (h + 1) * 64)
                            nc.tensor.matmul(
                                s_t[:, h * 512 : (h + 1) * 512],
                                lhsT=kT_sb[hp, b, ks],
                                rhs=qT_sb[hp, b, qs],
                                start=True,
                                stop=True,
                            )
                        stile[i] = s_t
                    if i >= SKEW:
                        j = i - SKEW
                        kt, b = subs[j]
                        pt = pt_pool.tile([128, 1024], BF, name="pt")
                        nc.scalar.activation(
                            out=pt,
                            in_=stile.pop(j),
                            func=mybir.ActivationFunctionType.Exp,
                        )
                        if kt % 4 == 3:
                            # every 4th bias-multiply on the otherwise-idle
                            # GPSIMD engine (SBUF-only operands qualify)
                            nc.gpsimd.tensor_mul(out=pt, in0=pt, in1=ebt[kt])
                        else:
                            nc.vector.tensor_mul(out=pt, in0=pt, in1=ebt[kt])
                        for h in range(HPC):
                            nc.tensor.matmul(
                                oacc[b][h],
                                lhsT=v_sb[:, b, kt, h * 128 : (h + 1) * 128],
                                rhs=pt[:, h * 512 : (h + 1) * 512],
                                start=(kt == 0),
                                stop=(kt == NKT - 1),
                            )
                        if kt == NKT - 1:
                            # sums final for (b, *): normalize now so the
                            # next qb's first AVs aren't blocked long
                            for h in range(HPC):
                                norm_chunk(qb, b, h, oacc[b][h])
                        # splice previous block's output projection
                        if pending and j % 2 == 0:
                            pending.pop(0)()
                while pending:
                    pending.pop(0)()
                pending = [
                    (lambda qb=qb, b=b, sti=sti, eb=eb: wo_chunk(qb, b, sti, eb))
                    for b in range(B)
                    for sti in range(4)
                    for eb in range(2)
                ]
            while pending:
                pending.pop(0)()


# ---------------------------------------------------------------------------
# Host side


def make_in_maps(
    hidden_states, bias, Wq, bq, Wk, bk, Wv, bv, Wo
) -> list[dict[str, np.ndarray]]:
    hidden_states = np.asarray(hidden_states, np.float32)
    bias = np.asarray(bias, np.float32)
    scale = 1.0 / np.sqrt(D)

    # shared across cores; [B, 128, ECH, S] partition-major for one-shot DMA
    xt = np.ascontiguousarray(
        hidden_states.transpose(0, 2, 1)  # [B, E, S]
        .reshape(B, ECH, 128, S)
        .transpose(0, 2, 1, 3)
    ).astype(BF16)

    in_maps = []
    for c in range(N_CORES):
        rows = slice(c * HPC * D, (c + 1) * HPC * D)  # 128 output dims
        wq_c = (np.asarray(Wq, np.float32)[rows, :] * scale).T  # [E, 128]
        wk_c = np.asarray(Wk, np.float32)[rows, :].T
        wv_c = np.asarray(Wv, np.float32)[rows, :].T
        bqkv_c = np.stack(
            [
                np.asarray(bq, np.float32)[rows] * scale,
                np.asarray(bk, np.float32)[rows],
                np.asarray(bv, np.float32)[rows],
            ],
            axis=1,
        )  # [128, 3]
        wo_c = np.asarray(Wo, np.float32)[:, rows].T  # [128, E]
        # [S(k), NQB, HPC, 512]: pbias[k, qb, h, q'] = exp(bias[0, h, qb*512+q', k])
        eb = np.exp(bias[0, c * HPC : (c + 1) * HPC])  # [HPC, Sq, Sk]
        pbias_c = np.ascontiguousarray(
            eb.reshape(HPC, NQB, 512, S).transpose(3, 1, 0, 2)
        )

        def pmajor(w):  # [E, 128] -> [128, ECH, 128] partition-major
            return np.ascontiguousarray(
                w.reshape(ECH, 128, 128).transpose(1, 0, 2)
            ).astype(BF16)

        in_maps.append(
            {
                "xt": xt,
                "wq": pmajor(wq_c),
                "wk": pmajor(wk_c),
                "wv": pmajor(wv_c),
                "bqkv": np.ascontiguousarray(bqkv_c),
                "wo": np.ascontiguousarray(wo_c).astype(BF16),
                "pbias": pbias_c.astype(BF16),
            }
        )
    return in_maps


_NC_CACHE: list = []
LAST_RESULTS = None


def kernel(hidden_states, bias, Wq, bq, Wk, bk, Wv, bv, Wo) -> np.ndarray:
    global LAST_RESULTS
    if not _NC_CACHE:
        _NC_CACHE.append(build_nc())
    nc = _NC_CACHE[0]
    in_maps = make_in_maps(hidden_states, bias, Wq, bq, Wk, bk, Wv, bv, Wo)
    res = run_bass_kernel_spmd(nc, in_maps, list(range(N_CORES)))
    LAST_RESULTS = res
    total = np.zeros((B, S, E), np.float32)
    for c in range(N_CORES):
        total += np.asarray(res.results[c]["out"], np.float32)
    return total


# revision 12
# speedup vs baseline: 1.1330x; 1.1330x over previous
"""AuroraAttention Trainium2 kernel — 8-core SPMD, head-sharded, v3.

Strategy (tensor parallel over heads, per sharding hint):
  - 16 heads -> 2 heads per core; both batches on every core.
  - Per core: q/k/v projections restricted to its 2 heads (column-parallel),
    full attention for its (batch, head) pairs, row-parallel output
    projection producing a partial [B, S, E] output; host sums the 8
    partials.
  - Scores are computed TRANSPOSED (S^T[k, q]) so the attention-weight
    matrix is already laid out with the contraction dim (k) on partitions
    for the A@V matmul. A 64-wide ones block in the V operand makes the
    same matmul produce the softmax denominators already broadcast across
    64 partitions.
  - softmax(s + b) = exp(s) * exp(b) with exp(b) precomputed on the host
    in bf16.

v3 performance structure (v1 ~383us, v2 ~404us):
  - ACT (scalar engine) is the long pole: the 16.8M-element exp stream is
    ~1ns/col + ~250-400ns fixed overhead per instruction.  So exps stay
    fused at [128, 1024] (both heads of one (kt, batch) pair -> 128 ops
    total), and EVERYTHING else is kept off ACT during attention.
  - Software pipeline with per-(kt,b) PSUM score tiles (2 banks, pool of
    2) and the drain chain (exp -> bias-multiply -> AV matmuls) emitted
    two sub-iterations behind the score matmuls: by drain time the
    dependencies have cleared, so the PE stream never parks on the
    serial exp->mul path (stalls also drop the PE out of its 2.4GHz
    p-state, which is what made v1/v2 matmuls ~1.7x slow).
  - 1/sumexp via DVE reciprocal_approx_fast (18-bit accurate, ~5x faster
    than the 4us InstReciprocal v1 used).
  - Every 4th bias-multiply runs on the otherwise-idle GPSIMD engine.
  - Output projection (one [128 s, 512 e] chunk at a time, PSUM tiles
    borrowed from the score pool, staged through SBUF on DVE) is
    deferred one q-block and spliced every other sub-iteration.
  - exp's ACT table is warmed up during the projection phase.
"""

import numpy as np
import ml_dtypes

import concourse.bass as bass
import concourse.mybir as mybir
import concourse.tile as tile
from concourse.bass_utils import run_bass_kernel_spmd
from concourse.masks import make_identity
from bass_rust import SyncInfo

BF16 = ml_dtypes.bfloat16
F32 = mybir.dt.float32
BF = mybir.dt.bfloat16

H, D, B, S, E = 16, 64, 2, 2048, 1024
N_CORES = 8
HPC = H // N_CORES  # heads per core
NQB = S // 512  # 4 q blocks
NKT = S // 128  # 16 k tiles
ECH = E // 128  # 8 contraction chunks for projections

SKEW = 2  # drain chain lags the score matmuls by this many (kt, b) sub-iters

# ---------------------------------------------------------------------------
# This walrus build rejects instructions carrying more than one sem wait
# ("Too many sync wait commands"). Tile freely emits multi-wait
# instructions, so after scheduling we move extra waits onto same-engine
# NoOps inserted immediately before the affected instruction. Engine
# streams execute in program order, so waiting on a preceding NoOp is
# semantically identical to waiting on the instruction itself.
_MAX_WAITS = 1


def split_multi_waits(nc: bass.Bass, max_waits: int = _MAX_WAITS):
    for bb in nc.main_func.blocks:
        lst = bb.instructions
        new = []
        changed = False
        for inst in lst:
            si = inst.sync_info
            if si is not None and si.on_wait and len(si.on_wait) > max_waits:
                waits = list(si.on_wait)
                extra, keep = waits[:-max_waits], waits[-max_waits:]
                for i in range(0, len(extra), max_waits):
                    nop = mybir.InstNoOp(
                        name=nc.get_next_instruction_name(), ins=[], outs=[]
                    )
                    nop.engine = inst.engine
                    nop.sync_info = SyncInfo(
                        on_wait=extra[i : i + max_waits], on_update=[]
                    )
                    nc.register_instruction(nop)
                    new.append(nop)
                inst.sync_info = SyncInfo(on_wait=keep, on_update=si.on_update)
                changed = True
            new.append(inst)
        if changed:
            bb.instructions = new
# ---------------------------------------------------------------------------


def build_nc() -> bass.Bass:
    nc = bass.Bass()

    # partition-major host layouts so each load is one big DMA
    xt = nc.dram_tensor("xt", [B, 128, ECH, S], BF, kind="ExternalInput")
    wq = nc.dram_tensor("wq", [128, ECH, 128], BF, kind="ExternalInput")
    wk = nc.dram_tensor("wk", [128, ECH, 128], BF, kind="ExternalInput")
    wv = nc.dram_tensor("wv", [128, ECH, 128], BF, kind="ExternalInput")
    bqkv = nc.dram_tensor("bqkv", [128, 3], F32, kind="ExternalInput")
    wo = nc.dram_tensor("wo", [128, E], BF, kind="ExternalInput")
    # exp(bias) transposed + host-packed so one [128, 1024] tile covering both
    # heads is one contiguous DMA: pbias[k, qb, h, q'] = exp(bias[0, h, qb*512+q', k])
    pbias = nc.dram_tensor("pbias", [S, NQB, HPC, 512], BF, kind="ExternalInput")
    out = nc.dram_tensor("out", [B, S, E], BF, kind="ExternalOutput")

    with tile.TileContext(nc) as tc:
        _emit(tc, nc, xt, wq, wk, wv, bqkv, wo, pbias, out)
    split_multi_waits(nc)
    return nc


def _emit(tc, nc, xt, wq, wk, wv, bqkv, wo, pbias, out):
    with tc.tile_pool(name="persist", bufs=1) as persist:
        # ---- persistent SBUF tensors -----------------------------------
        xt_sb = persist.tile([128, B, ECH, S], BF)  # hidden^T
        w_sb = persist.tile([128, 3, ECH, 128], BF)  # WqT/WkT/WvT chunks
        b_sb = persist.tile([128, 3], F32)  # bq/bk/bv (prescaled)
        wo_sb = persist.tile([128, E], BF)  # Wo slice^T, both heads
        qT_sb = persist.tile([128, B, S], BF)  # q^T (2 heads on partitions)
        kT_sb = persist.tile([128, B, S], BF)
        vT_sb = persist.tile([128, B, S], BF)  # v^T before transpose
        # v natural layout per k-tile: [v_h0 | ones64 | ones64 | v_h1]
        # -> AV matmul h0 gives O^T rows 0:64 + bcast sums rows 64:128;
        #    AV matmul h1 gives bcast sums rows 0:64 + O^T rows 64:128.
        v_sb = persist.tile([128, B, NKT, 256], BF)
        o_norm = persist.tile([128, B, S], BF)  # normalized O^T, both heads
        ident = persist.tile([128, 128], BF)
        warm = persist.tile([128, 1], F32)

        nc.vector.memset(v_sb[:, :, :, 64:192], 1.0)
        nc.vector.memset(warm, 0.0)
        # load exp's ACT table during the projection phase, not at the
        # first real exp
        nc.scalar.activation(
            out=warm, in_=warm, func=mybir.ActivationFunctionType.Exp
        )
        make_identity(nc, ident)

        # batched initial loads (host layouts are partition-major)
        for pi, w in enumerate((wq, wk, wv)):
            nc.sync.dma_start(out=w_sb[:, pi, :, :], in_=w[:, :, :])
        nc.sync.dma_start(out=b_sb, in_=bqkv[:, :])
        nc.sync.dma_start(out=wo_sb, in_=wo[:, :])
        for b in range(B):
            nc.sync.dma_start(out=xt_sb[:, b, :, :], in_=xt[b])

        # ---- projections ------------------------------------------------
        with (
            tc.tile_pool(name="proj_ps", bufs=2, space="PSUM") as proj_ps,
            tc.tile_pool(name="vtr_ps", bufs=2, space="PSUM") as vtr_ps,
        ):
            dsts = (qT_sb, kT_sb, vT_sb)
            for b in range(B):
                for pi in range(3):
                    for sblk in range(S // 512):
                        ps = proj_ps.tile([128, 512], F32, name="pps")
                        for c in range(ECH):
                            nc.tensor.matmul(
                                ps,
                                lhsT=w_sb[:, pi, c, :],
                                rhs=xt_sb[:, b, c, sblk * 512 : (sblk + 1) * 512],
                                start=(c == 0),
                                stop=(c == ECH - 1),
                            )
                        nc.scalar.activation(
                            out=dsts[pi][:, b, sblk * 512 : (sblk + 1) * 512],
                            in_=ps,
                            func=mybir.ActivationFunctionType.Identity,
                            bias=b_sb[:, pi : pi + 1],
                            scale=1.0,
                        )
                # v^T -> v natural (PE transpose per 128-wide s tile)
                for st in range(NKT):
                    tp = vtr_ps.tile([128, 128], BF, name="vtp")
                    nc.tensor.transpose(
                        out=tp,
                        in_=vT_sb[:, b, st * 128 : (st + 1) * 128],
                        identity=ident,
                    )
                    nc.scalar.copy(out=v_sb[:, b, st, 0:64], in_=tp[:, 0:64])
                    nc.scalar.copy(out=v_sb[:, b, st, 192:256], in_=tp[:, 64:128])

        # ---- software-pipelined attention -------------------------------
        # Sub-iteration = (kt, b): two K=64 score matmuls (row-packed heads)
        # into one 2-bank PSUM tile, one fused exp on ACT, one bias
        # multiply (DVE, every 4th on GPSIMD), two AV matmuls accumulating
        # into oacc[b][h].  The drain chain runs SKEW sub-iters behind the
        # score matmuls.  ps_pool (2 bufs x 2 banks) rotates score tiles
        # and the borrowed output-projection tiles; oacc holds the other
        # 4 banks.
        with (
            tc.tile_pool(name="eb_sb", bufs=4) as eb_pool,
            tc.tile_pool(name="pt_sb", bufs=4) as pt_pool,
            tc.tile_pool(name="norm_sb", bufs=4) as norm_pool,
            tc.tile_pool(name="wo_stage", bufs=4) as wo_stage,
            tc.tile_pool(name="ps_pool", bufs=2, space="PSUM") as ps_pool,
            tc.tile_pool(name="oacc_ps", bufs=1, space="PSUM") as oacc_ps,
        ):
            PREF = 3  # ebt DMA prefetch distance in kt units

            def norm_chunk(qb, b, h, oacc_t):
                # o_norm = O^T / sumexp; ones-block placement puts
                # h0: O^T rows 0:64, sums rows 64:128 (h1 mirrored).
                # 1/x on ACT as exp(-ln(x)): Ln and Exp share an act
                # table, and ACT has slack while DVE (where v1's 4us
                # reciprocal lived) is the attention-phase pacer.
                qs = slice(qb * 512, (qb + 1) * 512)
                o_rows = slice(0, 64) if h == 0 else slice(64, 128)
                s_rows = slice(64, 128) if h == 0 else slice(0, 64)
                ln_t = norm_pool.tile([128, 512], F32, name=f"ln{b}{h}")
                r_t = norm_pool.tile([128, 512], F32, name=f"r{b}{h}")
                nc.scalar.activation(
                    out=ln_t[s_rows, :],
                    in_=oacc_t[s_rows, :],
                    func=mybir.ActivationFunctionType.Ln,
                )
                nc.scalar.activation(
                    out=r_t[o_rows, :],
                    in_=ln_t[s_rows, :],
                    func=mybir.ActivationFunctionType.Exp,
                    scale=-1.0,
                )
                nc.vector.tensor_mul(
                    out=o_norm[o_rows, b, qs],
                    in0=oacc_t[o_rows, :],
                    in1=r_t[o_rows, :],
                )

            def wo_chunk(qb, b, sti, eb):
                # one [128 s, 512 e] slice of the output projection,
                # staged through SBUF on DVE (no PSUM->DRAM DMA on this
                # build)
                st = qb * 4 + sti
                ps = ps_pool.tile([128, 512], F32, name="ps")
                nc.tensor.matmul(
                    ps,
                    lhsT=o_norm[:, b, st * 128 : (st + 1) * 128],
                    rhs=wo_sb[:, eb * 512 : (eb + 1) * 512],
                    start=True,
                    stop=True,
                )
                stg = wo_stage.tile([128, 512], BF, name="stg")
                nc.vector.tensor_copy(out=stg, in_=ps)
                nc.sync.dma_start(
                    out=out[b, st * 128 : (st + 1) * 128, eb * 512 : (eb + 1) * 512],
                    in_=stg,
                )

            subs = [(kt, b) for kt in range(NKT) for b in range(B)]
            NSUB = len(subs)  # 32
            pending: list = []  # deferred wo chunks of the previous qb
            for qb in range(NQB):
                qs = slice(qb * 512, (qb + 1) * 512)
                oacc = [
                    [
                        oacc_ps.tile([128, 512], F32, name=f"oacc_{b}_{h}")
                        for h in range(HPC)
                    ]
                    for b in range(B)
                ]
                ebt = {}
                for kt in range(PREF):
                    t = eb_pool.tile([128, 1024], BF, name="ebt")
                    nc.sync.dma_start(out=t, in_=pbias[kt * 128 : (kt + 1) * 128, qb])
                    ebt[kt] = t
                stile = {}
                for i in range(NSUB + SKEW):
                    if i < NSUB:
                        kt, b = subs[i]
                        if b == 0 and kt + PREF < NKT:
                            pkt = kt + PREF
                            t = eb_pool.tile([128, 1024], BF, name="ebt")
                            nc.sync.dma_start(
                                out=t, in_=pbias[pkt * 128 : (pkt + 1) * 128, qb]
                            )
                            ebt[pkt] = t
                        ks = slice(kt * 128, (kt + 1) * 128)
                        s_t = ps_pool.tile([128, 1024], F32, name="ps")
                        for h in range(HPC):
                            hp = slice(h * 64, (h + 1) * 64)
                            nc.tensor.matmul(
                                s_t[:, h * 512 : (h + 1) * 512],
                                lhsT=kT_sb[hp, b, ks],
                                rhs=qT_sb[hp, b, qs],
                                start=True,
                                stop=True,
                            )
                        stile[i] = s_t
                    if i >= SKEW:
                        j = i - SKEW
                        kt, b = subs[j]
                        pt = pt_pool.tile([128, 1024], BF, name="pt")
                        nc.scalar.activation(
                            out=pt,
                            in_=stile.pop(j),
                            func=mybir.ActivationFunctionType.Exp,
                        )
                        nc.vector.tensor_mul(out=pt, in0=pt, in1=ebt[kt])
                        for h in range(HPC):
                            nc.tensor.matmul(
                                oacc[b][h],
                                lhsT=v_sb[:, b, kt, h * 128 : (h + 1) * 128],
                                rhs=pt[:, h * 512 : (h + 1) * 512],
                                start=(kt == 0),
                                stop=(kt == NKT - 1),
                            )
                        if kt == NKT - 1:
                            # sums final for (b, *): normalize now so the
                            # next qb's first AVs aren't blocked long
                            for h in range(HPC):
                                norm_chunk(qb, b, h, oacc[b][h])
                        # splice previous block's output projection
                        if pending and j % 2 == 0:
                            pending.pop(0)()
                while pending:
                    pending.pop(0)()
                pending = [
                    (lambda qb=qb, b=b, sti=sti, eb=eb: wo_chunk(qb, b, sti, eb))
                    for b in range(B)
                    for sti in range(4)
                    for eb in range(2)
                ]
            while pending:
                pending.pop(0)()


# ---------------------------------------------------------------------------
# Host side


def make_in_maps(
    hidden_states, bias, Wq, bq, Wk, bk, Wv, bv, Wo
) -> list[dict[str, np.ndarray]]:
    hidden_states = np.asarray(hidden_states, np.float32)
    bias = np.asarray(bias, np.float32)
    scale = 1.0 / np.sqrt(D)

    # shared across cores; [B, 128, ECH, S] partition-major for one-shot DMA
    xt = np.ascontiguousarray(
        hidden_states.transpose(0, 2, 1)  # [B, E, S]
        .reshape(B, ECH, 128, S)
        .transpose(0, 2, 1, 3)
    ).astype(BF16)

    in_maps = []
    for c in range(N_CORES):
        rows = slice(c * HPC * D, (c + 1) * HPC * D)  # 128 output dims
        wq_c = (np.asarray(Wq, np.float32)[rows, :] * scale).T  # [E, 128]
        wk_c = np.asarray(Wk, np.float32)[rows, :].T
        wv_c = np.asarray(Wv, np.float32)[rows, :].T
        bqkv_c = np.stack(
            [
                np.asarray(bq, np.float32)[rows] * scale,
                np.asarray(bk, np.float32)[rows],
                np.asarray(bv, np.float32)[rows],
            ],
            axis=1,
        )  # [128, 3]
        wo_c = np.asarray(Wo, np.float32)[:, rows].T  # [128, E]
        # [S(k), NQB, HPC, 512]: pbias[k, qb, h, q'] = exp(bias[0, h, qb*512+q', k])
        eb = np.exp(bias[0, c * HPC : (c + 1) * HPC])  # [HPC, Sq, Sk]
        pbias_c = np.ascontiguousarray(
            eb.reshape(HPC, NQB, 512, S).transpose(3, 1, 0, 2)
        )

        def pmajor(w):  # [E, 128] -> [128, ECH, 128] partition-major
            return np.ascontiguousarray(
                w.reshape(ECH, 128, 128).transpose(1, 0, 2)
            ).astype(BF16)

        in_maps.append(
            {
                "xt": xt,
                "wq": pmajor(wq_c),
                "wk": pmajor(wk_c),
                "wv": pmajor(wv_c),
                "bqkv": np.ascontiguousarray(bqkv_c),
                "wo": np.ascontiguousarray(wo_c).astype(BF16),
                "pbias": pbias_c.astype(BF16),
            }
        )
    return in_maps


_NC_CACHE: list = []
LAST_RESULTS = None


def kernel(hidden_states, bias, Wq, bq, Wk, bk, Wv, bv, Wo) -> np.ndarray:
    global LAST_RESULTS
    if not _NC_CACHE:
        _NC_CACHE.append(build_nc())
    nc = _NC_CACHE[0]
    in_maps = make_in_maps(hidden_states, bias, Wq, bq, Wk, bk, Wv, bv, Wo)
    res = run_bass_kernel_spmd(nc, in_maps, list(range(N_CORES)))
    LAST_RESULTS = res
    total = np.zeros((B, S, E), np.float32)
    for c in range(N_CORES):
        total += np.asarray(res.results[c]["out"], np.float32)
    return total


# revision 14
# speedup vs baseline: 1.2831x; 1.1325x over previous
"""AuroraAttention Trainium2 kernel — 8-core SPMD, head-sharded, v3.

Strategy (tensor parallel over heads, per sharding hint):
  - 16 heads -> 2 heads per core; both batches on every core.
  - Per core: q/k/v projections restricted to its 2 heads (column-parallel),
    full attention for its (batch, head) pairs, row-parallel output
    projection producing a partial [B, S, E] output; host sums the 8
    partials.
  - Scores are computed TRANSPOSED (S^T[k, q]) so the attention-weight
    matrix is already laid out with the contraction dim (k) on partitions
    for the A@V matmul. A 64-wide ones block in the V operand makes the
    same matmul produce the softmax denominators already broadcast across
    64 partitions.
  - softmax(s + b) = exp(s) * exp(b) with exp(b) precomputed on the host
    in bf16.

v3 performance structure (v1 ~383us, v2 ~404us):
  - ACT (scalar engine) is the long pole: the 16.8M-element exp stream is
    ~1ns/col + ~250-400ns fixed overhead per instruction.  So exps stay
    fused at [128, 1024] (both heads of one (kt, batch) pair -> 128 ops
    total), and EVERYTHING else is kept off ACT during attention.
  - Software pipeline with per-(kt,b) PSUM score tiles (2 banks, pool of
    2) and the drain chain (exp -> bias-multiply -> AV matmuls) emitted
    two sub-iterations behind the score matmuls: by drain time the
    dependencies have cleared, so the PE stream never parks on the
    serial exp->mul path (stalls also drop the PE out of its 2.4GHz
    p-state, which is what made v1/v2 matmuls ~1.7x slow).
  - 1/sumexp via DVE reciprocal_approx_fast (18-bit accurate, ~5x faster
    than the 4us InstReciprocal v1 used).
  - Every 4th bias-multiply runs on the otherwise-idle GPSIMD engine.
  - Output projection (one [128 s, 512 e] chunk at a time, PSUM tiles
    borrowed from the score pool, staged through SBUF on DVE) is
    deferred one q-block and spliced every other sub-iteration.
  - exp's ACT table is warmed up during the projection phase.
"""

import numpy as np
import ml_dtypes

import concourse.bass as bass
import concourse.mybir as mybir
import concourse.tile as tile
from concourse.bass_utils import run_bass_kernel_spmd
from concourse.masks import make_identity
from bass_rust import SyncInfo

BF16 = ml_dtypes.bfloat16
F32 = mybir.dt.float32
BF = mybir.dt.bfloat16

H, D, B, S, E = 16, 64, 2, 2048, 1024
N_CORES = 8
HPC = H // N_CORES  # heads per core
NQB = S // 512  # 4 q blocks
NKT = S // 128  # 16 k tiles
ECH = E // 128  # 8 contraction chunks for projections

SKEW = 2  # drain chain lags the score matmuls by this many (kt, b) sub-iters

# ---------------------------------------------------------------------------
# This walrus build rejects instructions carrying more than one sem wait
# ("Too many sync wait commands"). Tile freely emits multi-wait
# instructions, so after scheduling we move extra waits onto same-engine
# NoOps inserted immediately before the affected instruction. Engine
# streams execute in program order, so waiting on a preceding NoOp is
# semantically identical to waiting on the instruction itself.
_MAX_WAITS = 1


def split_multi_waits(nc: bass.Bass, max_waits: int = _MAX_WAITS):
    for bb in nc.main_func.blocks:
        lst = bb.instructions
        new = []
        changed = False
        for inst in lst:
            si = inst.sync_info
            if si is not None and si.on_wait and len(si.on_wait) > max_waits:
                waits = list(si.on_wait)
                extra, keep = waits[:-max_waits], waits[-max_waits:]
                for i in range(0, len(extra), max_waits):
                    nop = mybir.InstNoOp(
                        name=nc.get_next_instruction_name(), ins=[], outs=[]
                    )
                    nop.engine = inst.engine
                    nop.sync_info = SyncInfo(
                        on_wait=extra[i : i + max_waits], on_update=[]
                    )
                    nc.register_instruction(nop)
                    new.append(nop)
                inst.sync_info = SyncInfo(on_wait=keep, on_update=si.on_update)
                changed = True
            new.append(inst)
        if changed:
            bb.instructions = new
# ---------------------------------------------------------------------------


def build_nc() -> bass.Bass:
    nc = bass.Bass()

    # partition-major host layouts so each load is one big DMA
    xt = nc.dram_tensor("xt", [B, 128, ECH, S], BF, kind="ExternalInput")
    wq = nc.dram_tensor("wq", [128, ECH, 128], BF, kind="ExternalInput")
    wk = nc.dram_tensor("wk", [128, ECH, 128], BF, kind="ExternalInput")
    wv = nc.dram_tensor("wv", [128, ECH, 128], BF, kind="ExternalInput")
    bqkv = nc.dram_tensor("bqkv", [128, 3], F32, kind="ExternalInput")
    wo = nc.dram_tensor("wo", [128, E], BF, kind="ExternalInput")
    # exp(bias) transposed + host-packed so one [128, 1024] tile covering both
    # heads is one contiguous DMA: pbias[k, qb, h, q'] = exp(bias[0, h, qb*512+q', k])
    pbias = nc.dram_tensor("pbias", [S, NQB, HPC, 512], BF, kind="ExternalInput")
    out = nc.dram_tensor("out", [B, S, E], BF, kind="ExternalOutput")

    with tile.TileContext(nc) as tc:
        _emit(tc, nc, xt, wq, wk, wv, bqkv, wo, pbias, out)
    split_multi_waits(nc)
    return nc


def _emit(tc, nc, xt, wq, wk, wv, bqkv, wo, pbias, out):
    with tc.tile_pool(name="persist", bufs=1) as persist:
        # ---- persistent SBUF tensors -----------------------------------
        xt_sb = persist.tile([128, B, ECH, S], BF)  # hidden^T
        w_sb = persist.tile([128, 3, ECH, 128], BF)  # WqT/WkT/WvT chunks
        b_sb = persist.tile([128, 3], F32)  # bq/bk/bv (prescaled)
        wo_sb = persist.tile([128, E], BF)  # Wo slice^T, both heads
        qT_sb = persist.tile([128, B, S], BF)  # q^T (2 heads on partitions)
        kT_sb = persist.tile([128, B, S], BF)
        vT_sb = persist.tile([128, B, S], BF)  # v^T before transpose
        # v natural layout per k-tile: [v_h0 | ones64 | ones64 | v_h1]
        # -> AV matmul h0 gives O^T rows 0:64 + bcast sums rows 64:128;
        #    AV matmul h1 gives bcast sums rows 0:64 + O^T rows 64:128.
        v_sb = persist.tile([128, B, NKT, 256], BF)
        o_norm = persist.tile([128, B, S], BF)  # normalized O^T, both heads
        ident = persist.tile([128, 128], BF)
        warm = persist.tile([128, 1], F32)

        nc.vector.memset(v_sb[:, :, :, 64:192], 1.0)
        nc.vector.memset(warm, 0.0)
        # load exp's ACT table during the projection phase, not at the
        # first real exp
        nc.scalar.activation(
            out=warm, in_=warm, func=mybir.ActivationFunctionType.Exp
        )
        make_identity(nc, ident)

        # batched initial loads (host layouts are partition-major)
        for pi, w in enumerate((wq, wk, wv)):
            nc.sync.dma_start(out=w_sb[:, pi, :, :], in_=w[:, :, :])
        nc.sync.dma_start(out=b_sb, in_=bqkv[:, :])
        nc.sync.dma_start(out=wo_sb, in_=wo[:, :])
        for b in range(B):
            nc.sync.dma_start(out=xt_sb[:, b, :, :], in_=xt[b])

        # ---- projections ------------------------------------------------
        with (
            tc.tile_pool(name="proj_ps", bufs=2, space="PSUM") as proj_ps,
            tc.tile_pool(name="vtr_ps", bufs=2, space="PSUM") as vtr_ps,
        ):
            dsts = (qT_sb, kT_sb, vT_sb)
            for b in range(B):
                for pi in range(3):
                    for sblk in range(S // 512):
                        ps = proj_ps.tile([128, 512], F32, name="pps")
                        for c in range(ECH):
                            nc.tensor.matmul(
                                ps,
                                lhsT=w_sb[:, pi, c, :],
                                rhs=xt_sb[:, b, c, sblk * 512 : (sblk + 1) * 512],
                                start=(c == 0),
                                stop=(c == ECH - 1),
                            )
                        nc.scalar.activation(
                            out=dsts[pi][:, b, sblk * 512 : (sblk + 1) * 512],
                            in_=ps,
                            func=mybir.ActivationFunctionType.Identity,
                            bias=b_sb[:, pi : pi + 1],
                            scale=1.0,
                        )
                # v^T -> v natural (PE transpose per 128-wide s tile)
                for st in range(NKT):
                    tp = vtr_ps.tile([128, 128], BF, name="vtp")
                    nc.tensor.transpose(
                        out=tp,
                        in_=vT_sb[:, b, st * 128 : (st + 1) * 128],
                        identity=ident,
                    )
                    nc.scalar.copy(out=v_sb[:, b, st, 0:64], in_=tp[:, 0:64])
                    nc.scalar.copy(out=v_sb[:, b, st, 192:256], in_=tp[:, 64:128])

        # ---- software-pipelined attention -------------------------------
        # Sub-iteration = (kt, b): two K=64 score matmuls (row-packed heads)
        # into one 2-bank PSUM tile, one fused exp on ACT, one bias
        # multiply (DVE, every 4th on GPSIMD), two AV matmuls accumulating
        # into oacc[b][h].  The drain chain runs SKEW sub-iters behind the
        # score matmuls.  ps_pool (2 bufs x 2 banks) rotates score tiles
        # and the borrowed output-projection tiles; oacc holds the other
        # 4 banks.
        with (
            # all 16 ebt tiles of a qb stay alive (both batches read them)
            # plus one in flight for the next qb
            tc.tile_pool(name="eb_sb", bufs=17) as eb_pool,
            tc.tile_pool(name="pt_sb", bufs=4) as pt_pool,
            tc.tile_pool(name="norm_sb", bufs=1) as norm_pool,
            tc.tile_pool(name="wo_stage", bufs=4) as wo_stage,
            tc.tile_pool(name="ps_pool", bufs=2, space="PSUM") as ps_pool,
            tc.tile_pool(name="oacc_ps", bufs=1, space="PSUM") as oacc_ps,
        ):
            PREF = 3  # ebt DMA prefetch distance in kt units

            def norm_chunk(qb, b, h, oacc_t):
                # o_norm = O^T / sumexp; ones-block placement puts
                # h0: O^T rows 0:64, sums rows 64:128 (h1 mirrored).
                # Runs overlapped with the OTHER batch's attention phase,
                # so the 3.4us DVE reciprocal is off the critical path.
                qs = slice(qb * 512, (qb + 1) * 512)
                o_rows = slice(0, 64) if h == 0 else slice(64, 128)
                s_rows = slice(64, 128) if h == 0 else slice(0, 64)
                r_t = norm_pool.tile([128, 512], F32, name=f"r{b}{h}")
                nc.vector.reciprocal(out=r_t[o_rows, :], in_=oacc_t[s_rows, :])
                nc.vector.tensor_mul(
                    out=o_norm[o_rows, b, qs],
                    in0=oacc_t[o_rows, :],
                    in1=r_t[o_rows, :],
                )

            def wo_chunk(qb, b, sti):
                # one [128 s, 1024 e] row-block of the output projection,
                # staged through SBUF (no PSUM->DRAM DMA on this build)
                st = qb * 4 + sti
                ps = ps_pool.tile([128, 1024], F32, name="ps")
                for eb in range(2):
                    nc.tensor.matmul(
                        ps[:, eb * 512 : (eb + 1) * 512],
                        lhsT=o_norm[:, b, st * 128 : (st + 1) * 128],
                        rhs=wo_sb[:, eb * 512 : (eb + 1) * 512],
                        start=True,
                        stop=True,
                    )
                stg = wo_stage.tile([128, 1024], BF, name="stg")
                nc.vector.tensor_copy(out=stg, in_=ps)
                nc.sync.dma_start(
                    out=out[b, st * 128 : (st + 1) * 128, :], in_=stg
                )

            # batch-major: all of b=0's k tiles, then b=1's.  b0's
            # normalization + output projection then overlap b1's
            # attention phase (and b1's overlap the next qb's b0 phase),
            # so neither sits on a phase boundary.
            subs = [(kt, b) for b in range(B) for kt in range(NKT)]
            NSUB = len(subs)  # 32
            pending: list = []  # deferred norm/wo work
            for qb in range(NQB):
                qs = slice(qb * 512, (qb + 1) * 512)
                oacc = [
                    [
                        oacc_ps.tile([128, 512], F32, name=f"oacc_{b}_{h}")
                        for h in range(HPC)
                    ]
                    for b in range(B)
                ]
                # ebt tiles live for the whole qb (used by both batches)
                ebt = {}
                for kt in range(PREF):
                    t = eb_pool.tile([128, 1024], BF, name="ebt")
                    nc.sync.dma_start(out=t, in_=pbias[kt * 128 : (kt + 1) * 128, qb])
                    ebt[kt] = t
                stile = {}
                for i in range(NSUB + SKEW):
                    if i < NSUB:
                        kt, b = subs[i]
                        if b == 0 and kt + PREF < NKT:
                            pkt = kt + PREF
                            t = eb_pool.tile([128, 1024], BF, name="ebt")
                            nc.sync.dma_start(
                                out=t, in_=pbias[pkt * 128 : (pkt + 1) * 128, qb]
                            )
                            ebt[pkt] = t
                        ks = slice(kt * 128, (kt + 1) * 128)
                        s_t = ps_pool.tile([128, 1024], F32, name="ps")
                        for h in range(HPC):
                            hp = slice(h * 64, (h + 1) * 64)
                            nc.tensor.matmul(
                                s_t[:, h * 512 : (h + 1) * 512],
                                lhsT=kT_sb[hp, b, ks],
                                rhs=qT_sb[hp, b, qs],
                                start=True,
                                stop=True,
                            )
                        stile[i] = s_t
                    if i >= SKEW:
                        j = i - SKEW
                        kt, b = subs[j]
                        pt = pt_pool.tile([128, 1024], BF, name="pt")
                        nc.scalar.activation(
                            out=pt,
                            in_=stile.pop(j),
                            func=mybir.ActivationFunctionType.Exp,
                        )
                        nc.vector.tensor_mul(out=pt, in0=pt, in1=ebt[kt])
                        for h in range(HPC):
                            nc.tensor.matmul(
                                oacc[b][h],
                                lhsT=v_sb[:, b, kt, h * 128 : (h + 1) * 128],
                                rhs=pt[:, h * 512 : (h + 1) * 512],
                                start=(kt == 0),
                                stop=(kt == NKT - 1),
                            )
                        if kt == NKT - 1:
                            # batch b's accumulation is final: queue its
                            # normalization + output projection to splice
                            # into the following sub-iterations
                            pending.extend(
                                (lambda qb=qb, b=b, h=h, t=oacc[b][h]:
                                 norm_chunk(qb, b, h, t))
                                for h in range(HPC)
                            )
                            pending.extend(
                                (lambda qb=qb, b=b, sti=sti: wo_chunk(qb, b, sti))
                                for sti in range(4)
                            )
                        # splice deferred norm/wo work every other sub-iter
                        if pending and j % 2 == 0:
                            pending.pop(0)()
            while pending:
                pending.pop(0)()


# ---------------------------------------------------------------------------
# Host side


def make_in_maps(
    hidden_states, bias, Wq, bq, Wk, bk, Wv, bv, Wo
) -> list[dict[str, np.ndarray]]:
    hidden_states = np.asarray(hidden_states, np.float32)
    bias = np.asarray(bias, np.float32)
    scale = 1.0 / np.sqrt(D)

    # shared across cores; [B, 128, ECH, S] partition-major for one-shot DMA
    xt = np.ascontiguousarray(
        hidden_states.transpose(0, 2, 1)  # [B, E, S]
        .reshape(B, ECH, 128, S)
        .transpose(0, 2, 1, 3)
    ).astype(BF16)

    in_maps = []
    for c in range(N_CORES):
        rows = slice(c * HPC * D, (c + 1) * HPC * D)  # 128 output dims
        wq_c = (np.asarray(Wq, np.float32)[rows, :] * scale).T  # [E, 128]
        wk_c = np.asarray(Wk, np.float32)[rows, :].T
        wv_c = np.asarray(Wv, np.float32)[rows, :].T
        bqkv_c = np.stack(
            [
                np.asarray(bq, np.float32)[rows] * scale,
                np.asarray(bk, np.float32)[rows],
                np.asarray(bv, np.float32)[rows],
            ],
            axis=1,
        )  # [128, 3]
        wo_c = np.asarray(Wo, np.float32)[:, rows].T  # [128, E]
        # [S(k), NQB, HPC, 512]: pbias[k, qb, h, q'] = exp(bias[0, h, qb*512+q', k])
        eb = np.exp(bias[0, c * HPC : (c + 1) * HPC])  # [HPC, Sq, Sk]
        pbias_c = np.ascontiguousarray(
            eb.reshape(HPC, NQB, 512, S).transpose(3, 1, 0, 2)
        )

        def pmajor(w):  # [E, 128] -> [128, ECH, 128] partition-major
            return np.ascontiguousarray(
                w.reshape(ECH, 128, 128).transpose(1, 0, 2)
            ).astype(BF16)

        in_maps.append(
            {
                "xt": xt,
                "wq": pmajor(wq_c),
                "wk": pmajor(wk_c),
                "wv": pmajor(wv_c),
                "bqkv": np.ascontiguousarray(bqkv_c),
                "wo": np.ascontiguousarray(wo_c).astype(BF16),
                "pbias": pbias_c.astype(BF16),
            }
        )
    return in_maps


_NC_CACHE: list = []
LAST_RESULTS = None


def kernel(hidden_states, bias, Wq, bq, Wk, bk, Wv, bv, Wo) -> np.ndarray:
    global LAST_RESULTS
    if not _NC_CACHE:
        _NC_CACHE.append(build_nc())
    nc = _NC_CACHE[0]
    in_maps = make_in_maps(hidden_states, bias, Wq, bq, Wk, bk, Wv, bv, Wo)
    res = run_bass_kernel_spmd(nc, in_maps, list(range(N_CORES)))
    LAST_RESULTS = res
    total = np.zeros((B, S, E), np.float32)
    for c in range(N_CORES):
        total += np.asarray(res.results[c]["out"], np.float32)
    return total


# revision 16
# speedup vs baseline: 1.4174x; 1.1047x over previous
"""AuroraAttention Trainium2 kernel — 8-core SPMD, head-sharded, v3.

Strategy (tensor parallel over heads, per sharding hint):
  - 16 heads -> 2 heads per core; both batches on every core.
  - Per core: q/k/v projections restricted to its 2 heads (column-parallel),
    full attention for its (batch, head) pairs, row-parallel output
    projection producing a partial [B, S, E] output; host sums the 8
    partials.
  - Scores are computed TRANSPOSED (S^T[k, q]) so the attention-weight
    matrix is already laid out with the contraction dim (k) on partitions
    for the A@V matmul. A 64-wide ones block in the V operand makes the
    same matmul produce the softmax denominators already broadcast across
    64 partitions.
  - softmax(s + b) = exp(s) * exp(b) with exp(b) precomputed on the host
    in bf16.

v3 performance structure (v1 ~383us, v2 ~404us):
  - ACT (scalar engine) is the long pole: the 16.8M-element exp stream is
    ~1ns/col + ~250-400ns fixed overhead per instruction.  So exps stay
    fused at [128, 1024] (both heads of one (kt, batch) pair -> 128 ops
    total), and EVERYTHING else is kept off ACT during attention.
  - Software pipeline with per-(kt,b) PSUM score tiles (2 banks, pool of
    2) and the drain chain (exp -> bias-multiply -> AV matmuls) emitted
    two sub-iterations behind the score matmuls: by drain time the
    dependencies have cleared, so the PE stream never parks on the
    serial exp->mul path (stalls also drop the PE out of its 2.4GHz
    p-state, which is what made v1/v2 matmuls ~1.7x slow).
  - 1/sumexp via DVE reciprocal_approx_fast (18-bit accurate, ~5x faster
    than the 4us InstReciprocal v1 used).
  - Every 4th bias-multiply runs on the otherwise-idle GPSIMD engine.
  - Output projection (one [128 s, 512 e] chunk at a time, PSUM tiles
    borrowed from the score pool, staged through SBUF on DVE) is
    deferred one q-block and spliced every other sub-iteration.
  - exp's ACT table is warmed up during the projection phase.
"""

import numpy as np
import ml_dtypes

import concourse.bass as bass
import concourse.mybir as mybir
import concourse.tile as tile
from concourse.bass_utils import run_bass_kernel_spmd
from concourse.masks import make_identity
from bass_rust import SyncInfo

BF16 = ml_dtypes.bfloat16
F32 = mybir.dt.float32
BF = mybir.dt.bfloat16

H, D, B, S, E = 16, 64, 2, 2048, 1024
N_CORES = 8
HPC = H // N_CORES  # heads per core
NQB = S // 512  # 4 q blocks
NKT = S // 128  # 16 k tiles
ECH = E // 128  # 8 contraction chunks for projections

SKEW = 2  # drain chain lags the score matmuls by this many (kt, b) sub-iters

# ---------------------------------------------------------------------------
# This walrus build rejects instructions carrying more than one sem wait
# ("Too many sync wait commands"). Tile freely emits multi-wait
# instructions, so after scheduling we move extra waits onto same-engine
# NoOps inserted immediately before the affected instruction. Engine
# streams execute in program order, so waiting on a preceding NoOp is
# semantically identical to waiting on the instruction itself.
_MAX_WAITS = 1


def split_multi_waits(nc: bass.Bass, max_waits: int = _MAX_WAITS):
    for bb in nc.main_func.blocks:
        lst = bb.instructions
        new = []
        changed = False
        for inst in lst:
            si = inst.sync_info
            if si is not None and si.on_wait and len(si.on_wait) > max_waits:
                waits = list(si.on_wait)
                extra, keep = waits[:-max_waits], waits[-max_waits:]
                for i in range(0, len(extra), max_waits):
                    nop = mybir.InstNoOp(
                        name=nc.get_next_instruction_name(), ins=[], outs=[]
                    )
                    nop.engine = inst.engine
                    nop.sync_info = SyncInfo(
                        on_wait=extra[i : i + max_waits], on_update=[]
                    )
                    nc.register_instruction(nop)
                    new.append(nop)
                inst.sync_info = SyncInfo(on_wait=keep, on_update=si.on_update)
                changed = True
            new.append(inst)
        if changed:
            bb.instructions = new
# ---------------------------------------------------------------------------


def build_nc() -> bass.Bass:
    nc = bass.Bass()

    # partition-major host layouts so each load is one big DMA
    xt = nc.dram_tensor("xt", [B, 128, ECH, S], BF, kind="ExternalInput")
    wq = nc.dram_tensor("wq", [128, ECH, 128], BF, kind="ExternalInput")
    wk = nc.dram_tensor("wk", [128, ECH, 128], BF, kind="ExternalInput")
    wv = nc.dram_tensor("wv", [128, ECH, 128], BF, kind="ExternalInput")
    bqkv = nc.dram_tensor("bqkv", [128, 3], F32, kind="ExternalInput")
    wo = nc.dram_tensor("wo", [128, E], BF, kind="ExternalInput")
    # exp(bias) transposed + host-packed so one [128, 1024] tile covering both
    # heads is one contiguous DMA: pbias[k, qb, h, q'] = exp(bias[0, h, qb*512+q', k])
    pbias = nc.dram_tensor("pbias", [S, NQB, HPC, 512], BF, kind="ExternalInput")
    out = nc.dram_tensor("out", [B, S, E], BF, kind="ExternalOutput")

    with tile.TileContext(nc) as tc:
        _emit(tc, nc, xt, wq, wk, wv, bqkv, wo, pbias, out)
    split_multi_waits(nc)
    return nc


def _emit(tc, nc, xt, wq, wk, wv, bqkv, wo, pbias, out):
    with tc.tile_pool(name="persist", bufs=1) as persist:
        # ---- persistent SBUF tensors -----------------------------------
        xt_sb = persist.tile([128, B, ECH, S], BF)  # hidden^T
        w_sb = persist.tile([128, 3, ECH, 128], BF)  # WqT/WkT/WvT chunks
        b_sb = persist.tile([128, 3], F32)  # bq/bk/bv (prescaled)
        wo_sb = persist.tile([128, E], BF)  # Wo slice^T, both heads
        qT_sb = persist.tile([128, B, S], BF)  # q^T (2 heads on partitions)
        kT_sb = persist.tile([128, B, S], BF)
        vT_sb = persist.tile([128, B, S], BF)  # v^T before transpose
        # v natural layout per k-tile: [v_h0 | ones64 | ones64 | v_h1]
        # -> AV matmul h0 gives O^T rows 0:64 + bcast sums rows 64:128;
        #    AV matmul h1 gives bcast sums rows 0:64 + O^T rows 64:128.
        v_sb = persist.tile([128, B, NKT, 256], BF)
        o_norm = persist.tile([128, B, S], BF)  # normalized O^T, both heads
        ident = persist.tile([128, 128], BF)
        warm = persist.tile([128, 1], F32)

        nc.vector.memset(v_sb[:, :, :, 64:192], 1.0)
        nc.vector.memset(warm, 0.0)
        # load exp's ACT table during the projection phase, not at the
        # first real exp
        nc.scalar.activation(
            out=warm, in_=warm, func=mybir.ActivationFunctionType.Exp
        )
        make_identity(nc, ident)

        # batched initial loads (host layouts are partition-major)
        for pi, w in enumerate((wq, wk, wv)):
            nc.sync.dma_start(out=w_sb[:, pi, :, :], in_=w[:, :, :])
        nc.sync.dma_start(out=b_sb, in_=bqkv[:, :])
        nc.sync.dma_start(out=wo_sb, in_=wo[:, :])
        # xt in ECH-pair chunks so the first projection matmuls can start
        # before the full 8.4MB hidden-state load lands
        for b in range(B):
            for ch in range(0, ECH, 2):
                nc.sync.dma_start(
                    out=xt_sb[:, b, ch : ch + 2, :], in_=xt[b, :, ch : ch + 2, :]
                )

        # ---- projections ------------------------------------------------
        with (
            tc.tile_pool(name="proj_ps", bufs=4, space="PSUM") as proj_ps,
            tc.tile_pool(name="vtr_ps", bufs=2, space="PSUM") as vtr_ps,
        ):
            dsts = (qT_sb, kT_sb, vT_sb)
            for b in range(B):
                for pi in range(3):
                    for sblk in range(S // 512):
                        ps = proj_ps.tile([128, 512], F32, name="pps")
                        for c in range(ECH):
                            nc.tensor.matmul(
                                ps,
                                lhsT=w_sb[:, pi, c, :],
                                rhs=xt_sb[:, b, c, sblk * 512 : (sblk + 1) * 512],
                                start=(c == 0),
                                stop=(c == ECH - 1),
                            )
                        nc.scalar.activation(
                            out=dsts[pi][:, b, sblk * 512 : (sblk + 1) * 512],
                            in_=ps,
                            func=mybir.ActivationFunctionType.Identity,
                            bias=b_sb[:, pi : pi + 1],
                            scale=1.0,
                        )
                # v^T -> v natural (PE transpose per 128-wide s tile)
                for st in range(NKT):
                    tp = vtr_ps.tile([128, 128], BF, name="vtp")
                    nc.tensor.transpose(
                        out=tp,
                        in_=vT_sb[:, b, st * 128 : (st + 1) * 128],
                        identity=ident,
                    )
                    nc.scalar.copy(out=v_sb[:, b, st, 0:64], in_=tp[:, 0:64])
                    nc.scalar.copy(out=v_sb[:, b, st, 192:256], in_=tp[:, 64:128])

        # ---- software-pipelined attention -------------------------------
        # Sub-iteration = (kt, b): two K=64 score matmuls (row-packed heads)
        # into one 2-bank PSUM tile, one fused exp on ACT, one bias
        # multiply (DVE, every 4th on GPSIMD), two AV matmuls accumulating
        # into oacc[b][h].  The drain chain runs SKEW sub-iters behind the
        # score matmuls.  ps_pool (2 bufs x 2 banks) rotates score tiles
        # and the borrowed output-projection tiles; oacc holds the other
        # 4 banks.
        with (
            # all 16 ebt tiles of a qb stay alive (both batches read them)
            # plus one in flight for the next qb
            tc.tile_pool(name="eb_sb", bufs=17) as eb_pool,
            tc.tile_pool(name="pt_sb", bufs=4) as pt_pool,
            tc.tile_pool(name="norm_sb", bufs=1) as norm_pool,
            tc.tile_pool(name="wo_stage", bufs=4) as wo_stage,
            tc.tile_pool(name="ps_pool", bufs=2, space="PSUM") as ps_pool,
            tc.tile_pool(name="oacc_ps", bufs=1, space="PSUM") as oacc_ps,
        ):
            PREF = 3  # ebt DMA prefetch distance in kt units

            def norm_chunk(qb, b, h, oacc_t):
                # o_norm = O^T / sumexp; ones-block placement puts
                # h0: O^T rows 0:64, sums rows 64:128 (h1 mirrored).
                # 1/x as exp(-ln(x)) on ACT: Ln/Exp share an act table,
                # ACT has attention-phase slack (DVE is the pacer), and
                # batch-major splicing keeps this chain off any phase
                # boundary.
                qs = slice(qb * 512, (qb + 1) * 512)
                o_rows = slice(0, 64) if h == 0 else slice(64, 128)
                s_rows = slice(64, 128) if h == 0 else slice(0, 64)
                ln_t = norm_pool.tile([128, 512], F32, name=f"ln{b}{h}")
                r_t = norm_pool.tile([128, 512], F32, name=f"r{b}{h}")
                nc.scalar.activation(
                    out=ln_t[s_rows, :],
                    in_=oacc_t[s_rows, :],
                    func=mybir.ActivationFunctionType.Ln,
                )
                nc.scalar.activation(
                    out=r_t[o_rows, :],
                    in_=ln_t[s_rows, :],
                    func=mybir.ActivationFunctionType.Exp,
                    scale=-1.0,
                )
                nc.vector.tensor_mul(
                    out=o_norm[o_rows, b, qs],
                    in0=oacc_t[o_rows, :],
                    in1=r_t[o_rows, :],
                )

            def wo_chunk(qb, b, sti):
                # one [128 s, 1024 e] row-block of the output projection,
                # staged through SBUF (no PSUM->DRAM DMA on this build)
                st = qb * 4 + sti
                ps = ps_pool.tile([128, 1024], F32, name="ps")
                for eb in range(2):
                    nc.tensor.matmul(
                        ps[:, eb * 512 : (eb + 1) * 512],
                        lhsT=o_norm[:, b, st * 128 : (st + 1) * 128],
                        rhs=wo_sb[:, eb * 512 : (eb + 1) * 512],
                        start=True,
                        stop=True,
                    )
                stg = wo_stage.tile([128, 1024], BF, name="stg")
                nc.vector.tensor_copy(out=stg, in_=ps)
                nc.sync.dma_start(
                    out=out[b, st * 128 : (st + 1) * 128, :], in_=stg
                )

            # batch-major: all of b=0's k tiles, then b=1's.  b0's
            # normalization + output projection then overlap b1's
            # attention phase (and b1's overlap the next qb's b0 phase),
            # so neither sits on a phase boundary.
            subs = [(kt, b) for b in range(B) for kt in range(NKT)]
            NSUB = len(subs)  # 32
            pending: list = []  # deferred norm/wo work
            for qb in range(NQB):
                qs = slice(qb * 512, (qb + 1) * 512)
                oacc = [
                    [
                        oacc_ps.tile([128, 512], F32, name=f"oacc_{b}_{h}")
                        for h in range(HPC)
                    ]
                    for b in range(B)
                ]
                # ebt tiles live for the whole qb (used by both batches)
                ebt = {}
                for kt in range(PREF):
                    t = eb_pool.tile([128, 1024], BF, name="ebt")
                    nc.sync.dma_start(out=t, in_=pbias[kt * 128 : (kt + 1) * 128, qb])
                    ebt[kt] = t
                stile = {}
                for i in range(NSUB + SKEW):
                    if i < NSUB:
                        kt, b = subs[i]
                        if b == 0 and kt + PREF < NKT:
                            pkt = kt + PREF
                            t = eb_pool.tile([128, 1024], BF, name="ebt")
                            nc.sync.dma_start(
                                out=t, in_=pbias[pkt * 128 : (pkt + 1) * 128, qb]
                            )
                            ebt[pkt] = t
                        ks = slice(kt * 128, (kt + 1) * 128)
                        s_t = ps_pool.tile([128, 1024], F32, name="ps")
                        for h in range(HPC):
                            hp = slice(h * 64, (h + 1) * 64)
                            nc.tensor.matmul(
                                s_t[:, h * 512 : (h + 1) * 512],
                                lhsT=kT_sb[hp, b, ks],
                                rhs=qT_sb[hp, b, qs],
                                start=True,
                                stop=True,
                            )
                        stile[i] = s_t
                    if i >= SKEW:
                        j = i - SKEW
                        kt, b = subs[j]
                        pt = pt_pool.tile([128, 1024], BF, name="pt")
                        nc.scalar.activation(
                            out=pt,
                            in_=stile.pop(j),
                            func=mybir.ActivationFunctionType.Exp,
                        )
                        nc.vector.tensor_mul(out=pt, in0=pt, in1=ebt[kt])
                        for h in range(HPC):
                            nc.tensor.matmul(
                                oacc[b][h],
                                lhsT=v_sb[:, b, kt, h * 128 : (h + 1) * 128],
                                rhs=pt[:, h * 512 : (h + 1) * 512],
                                start=(kt == 0),
                                stop=(kt == NKT - 1),
                            )
                        if kt == NKT - 1:
                            # batch b's accumulation is final: queue its
                            # normalization + output projection to splice
                            # into the following sub-iterations
                            pending.extend(
                                (lambda qb=qb, b=b, h=h, t=oacc[b][h]:
                                 norm_chunk(qb, b, h, t))
                                for h in range(HPC)
                            )
                            pending.extend(
                                (lambda qb=qb, b=b, sti=sti: wo_chunk(qb, b, sti))
                                for sti in range(4)
                            )
                        # splice deferred norm/wo work every other sub-iter
                        if pending and j % 2 == 0:
                            pending.pop(0)()
            while pending:
                pending.pop(0)()


# ---------------------------------------------------------------------------
# Host side


def make_in_maps(
    hidden_states, bias, Wq, bq, Wk, bk, Wv, bv, Wo
) -> list[dict[str, np.ndarray]]:
    hidden_states = np.asarray(hidden_states, np.float32)
    bias = np.asarray(bias, np.float32)
    scale = 1.0 / np.sqrt(D)

    # shared across cores; [B, 128, ECH, S] partition-major for one-shot DMA
    xt = np.ascontiguousarray(
        hidden_states.transpose(0, 2, 1)  # [B, E, S]
        .reshape(B, ECH, 128, S)
        .transpose(0, 2, 1, 3)
    ).astype(BF16)

    in_maps = []
    for c in range(N_CORES):
        rows = slice(c * HPC * D, (c + 1) * HPC * D)  # 128 output dims
        wq_c = (np.asarray(Wq, np.float32)[rows, :] * scale).T  # [E, 128]
        wk_c = np.asarray(Wk, np.float32)[rows, :].T
        wv_c = np.asarray(Wv, np.float32)[rows, :].T
        bqkv_c = np.stack(
            [
                np.asarray(bq, np.float32)[rows] * scale,
                np.asarray(bk, np.float32)[rows],
                np.asarray(bv, np.float32)[rows],
            ],
            axis=1,
        )  # [128, 3]
        wo_c = np.asarray(Wo, np.float32)[:, rows].T  # [128, E]
        # [S(k), NQB, HPC, 512]: pbias[k, qb, h, q'] = exp(bias[0, h, qb*512+q', k])
        eb = np.exp(bias[0, c * HPC : (c + 1) * HPC])  # [HPC, Sq, Sk]
        pbias_c = np.ascontiguousarray(
            eb.reshape(HPC, NQB, 512, S).transpose(3, 1, 0, 2)
        )

        def pmajor(w):  # [E, 128] -> [128, ECH, 128] partition-major
            return np.ascontiguousarray(
                w.reshape(ECH, 128, 128).transpose(1, 0, 2)
            ).astype(BF16)

        in_maps.append(
            {
                "xt": xt,
                "wq": pmajor(wq_c),
                "wk": pmajor(wk_c),
                "wv": pmajor(wv_c),
                "bqkv": np.ascontiguousarray(bqkv_c),
                "wo": np.ascontiguousarray(wo_c).astype(BF16),
                "pbias": pbias_c.astype(BF16),
            }
        )
    return in_maps


_NC_CACHE: list = []
LAST_RESULTS = None


def kernel(hidden_states, bias, Wq, bq, Wk, bk, Wv, bv, Wo) -> np.ndarray:
    global LAST_RESULTS
    if not _NC_CACHE:
        _NC_CACHE.append(build_nc())
    nc = _NC_CACHE[0]
    in_maps = make_in_maps(hidden_states, bias, Wq, bq, Wk, bk, Wv, bv, Wo)
    res = run_bass_kernel_spmd(nc, in_maps, list(range(N_CORES)))
    LAST_RESULTS = res
    total = np.zeros((B, S, E), np.float32)
    for c in range(N_CORES):
        total += np.asarray(res.results[c]["out"], np.float32)
    return total
